# revision 38
# baseline (speedup 1.0000x reference)
"""KeypointFlowLoss Trainium2 kernel.

The loss only reads each flow at the K keypoint pixels the reference
scatters into the ground-truth image (everywhere else gt == 0, mask == 0),
so instead of streaming 5 x [16,2,512,512] f32 from HBM we gather exactly
the needed pixels with one indirect DMA per core and reduce on-chip.

Sharding: data-parallel over batch — core c owns batches [2c, 2c+2).
Host-side marshalling re-lays the five flows out as one [B,H,W,5,2] tensor
(per-core slice is a contiguous view) so each keypoint's (x,y) pair for
one flow is 8 contiguous bytes, and packs per core one [5,102] i32 aux
block: cols 0-33 the gather's pair-index table (row f = flow f's pair for
each keypoint), cols 34-101 the bitcast f32 keypoint displacements.
Masked-out keypoints get out-of-bounds indices (silently dropped by the
gather, leaving memset zeros) and zero disp, so they contribute exactly 0
to every sum with no mask multiply.

The program is raw bass (no TileContext): hand-placed semaphores, no
entry/exit all-engine drain barriers, which saves ~0.8us of scaffold.
Device critical path per core (everything else overlaps):
  aux DMA -> indirect gather g[5,68] -> d=g-disp, d^2, pair-sum [5,34]
  -> ACT sqrt with free-axis accumulate -> [5,1] partial sums -> out DMA.
The host adds the 8 cores' partials, divides by the host-computed mask
count, and applies the gamma weighting.
"""

import numpy as np

import concourse.bacc as bacc
import concourse.bass as bass
import concourse.mybir as mybir
from concourse.bass import IndirectOffsetOnAxis
from concourse.bass_utils import run_bass_kernel_spmd

B, CH, H, W = 16, 2, 512, 512
K = 17
NF = 5
NCORES = 8
BL = B // NCORES          # batches per core
NP = BL * K               # keypoints per core
NV = NF * CH              # flow values per keypoint
TOT = BL * H * W * NV     # per-core flow elements
GAMMA = 0.8
LOSS_WEIGHT = 1.0

F32 = mybir.dt.float32
I32 = mybir.dt.int32

_PROGRAM = None
_RUN_KWARGS = {}      # test harness can set {"trace": True} to profile
_LAST_RESULTS = None


def _build_program():
    """Raw-bass program (no TileContext): hand-placed semaphores, no entry or
    exit all-engine barriers. Every cross-engine edge is a producer .then_inc
    paired with a consumer wait that also decrements, so all semaphores read
    zero again at program end and the NEFF is safely re-executable."""
    nc = bacc.Bacc(None, target_bir_lowering=False)

    # Strip the constructor's entry all-engine barrier: every cross-engine
    # edge below carries an explicit semaphore, and the const-AP pool the
    # barrier protects is not used (the activation bias is an own memset
    # tile synced through the s_v chain).
    entry = nc.main_func.blocks[0]
    for inst in [i for i in entry.instructions
                 if isinstance(i, (mybir.InstDrain, mybir.InstEventSemaphore))]:
        entry.instructions.remove(inst)

    fs = nc.dram_tensor("fs", [TOT], F32, kind="ExternalInput")
    aux = nc.dram_tensor("aux", [NF, 3 * NP], I32, kind="ExternalInput")
    out = nc.dram_tensor("out", [NF, 1], F32, kind="ExternalOutput")

    s_rdy = nc.alloc_semaphore("s_rdy")  # aux table (+16) and g memset (+1)
    s_g = nc.alloc_semaphore("s_g")     # gather landed
    s_o = nc.alloc_semaphore("s_o")     # out DMA landed
    s_v = nc.alloc_semaphore("s_v")     # compute chain progress counter
    sems = [s_rdy, s_g, s_o, s_v]

    with (
        nc.sbuf_tensor([NF, 3 * NP], I32) as at,
        nc.sbuf_tensor([NF, 2 * NP], F32) as g,
        nc.sbuf_tensor([NF, 2 * NP], F32) as d,
        nc.sbuf_tensor([NF, NP], F32) as s,
        nc.sbuf_tensor([NF, NP], F32) as epe,
        nc.sbuf_tensor([NF, 1], F32) as res,
        nc.sbuf_tensor([NF, 1], F32) as zb,
    ):
        # cols 0-33: gather index table; cols 34-101: bitcast f32 disp
        nc.sync.dma_start(out=at[:], in_=aux[:]).then_inc(s_rdy, 16)
        disp = at[:, NP:3 * NP].bitcast(F32)

        nc.vector.memset(zb[:], 0.0)   # activation bias zeros (synced via s_v)
        nc.vector.memset(g[:], 0.0).then_inc(s_rdy, 1)

        # gather: each index fetches the contiguous (x,y) f32 pair of one
        # flow at one keypoint; OOB indices (masked keypoints) are dropped
        # and leave the memset zeros in place. g[f, 2i+c] = value.
        flat = bass.AP(fs, 0, [[2, TOT // 2], [1, 2]])
        nc.gpsimd.indirect_dma_start(
            out=g[:],
            out_offset=None,
            in_=flat,
            in_offset=IndirectOffsetOnAxis(ap=at[:, 0:NP], axis=0),
            bounds_check=TOT // 2 - 1,
            oob_is_err=False,
        ).wait_op(s_rdy, 17, "sem-ge").then_inc(s_g, 16)

        # col 2i: x-diff of keypoint i; col 2i+1: y-diff
        # Semaphore decrements ride on instructions that retire strictly
        # after the protected data's last reader, restoring every semaphore
        # to zero by program end (NEFF re-execution safety).
        nc.vector.tensor_tensor(out=d[:], in0=g[:], in1=disp,
                                op=mybir.AluOpType.subtract) \
            .wait_op(s_g, 16, "sem-ge").then_inc(s_v, 1)
        nc.vector.tensor_tensor(out=d[:], in0=d[:], in1=d[:],
                                op=mybir.AluOpType.mult) \
            .wait_op(s_v, 1, "sem-ge").then_inc(s_v, 1)
        nc.vector.tensor_tensor(out=s[:], in0=d[:, 0:2 * NP:2],
                                in1=d[:, 1:2 * NP:2],
                                op=mybir.AluOpType.add) \
            .wait_op(s_v, 2, "sem-ge").then_inc(s_v, 1)

        # epe = sqrt(s); accum_out gives the per-flow keypoint sum
        nc.scalar.activation(out=epe[:], in_=s[:],
                             func=mybir.ActivationFunctionType.Sqrt,
                             bias=zb[:], accum_out=res[:]) \
            .wait_op(s_v, 3, "sem-ge").then_inc(s_v, 1)

        nc.sync.dma_start(out=out[:], in_=res[:]) \
            .wait_op(s_v, 4, "sem-ge").then_inc(s_o, 16)
        # one barrier so every engine's updates retire, then a single
        # range-clear resets the semaphores for NEFF re-execution
        nc.sync.wait_ge(s_o, 16)
        nc.all_engine_barrier(sem_only=True)
        lo = min(sm.num for sm in sems)
        hi = max(sm.num for sm in sems)
        nc.sync.sem_clear(range(lo, hi + 1))

    nc.finalize()
    return nc


def _get_program():
    global _PROGRAM
    if _PROGRAM is None:
        _PROGRAM = _build_program()
    return _PROGRAM


def _shard_inputs(inputs):
    """Host-side marshalling: returns (in_maps for the 8 cores, mask count)."""
    flows = [np.asarray(inputs[f"flow{i}"], dtype=np.float32) for i in range(NF)]
    kps = np.asarray(inputs["kps"], dtype=np.int64)

    # T[b,y,x,f,c] = flow_f[b,c,y,x]; per-core slice stays a contiguous view.
    t = np.ascontiguousarray(
        np.stack(flows, axis=0).transpose(1, 3, 4, 0, 2)
    ).reshape(B, H * W * NV)

    kps0, kps1 = kps[:, 0], kps[:, 1]        # [B, K, 2] (x, y)
    x0, y0 = kps0[..., 0], kps0[..., 1]
    x1, y1 = kps1[..., 0], kps1[..., 1]
    valid = (
        (kps0 >= 0).all(-1) & (kps1 >= 0).all(-1)
        & (x0 < W) & (y0 < H) & (x1 < W) & (y1 < H)
    )
    disp = (kps1 - kps0).astype(np.float32)  # [B, K, 2]
    mask = valid & (kps1 != kps0).any(-1)    # [B, K]
    disp[~mask] = 0.0

    # pair index of (b, y0, x0)'s first flow pair; OOB when masked out
    idx = np.where(mask, (y0 * W + x0) * NF, TOT).astype(np.int64)   # [B, K]

    in_maps = []
    for c in range(NCORES):
        sl = slice(c * BL, (c + 1) * BL)
        loc = idx[sl] + (np.arange(BL) * (H * W * NF))[:, None]   # [BL, K]
        aux = np.empty((NF, 3 * NP), dtype=np.int32)
        # cols 0-33: pair-index of flow f's (x,y) pair of keypoint i at [f, i]
        f_off = np.arange(NF, dtype=np.int64)[:, None]            # [NF, 1]
        aux[:, 0:NP] = (loc.reshape(1, NP) + f_off).astype(np.int32)
        # cols 34-101: bitcast f32 disp, (dx_i, dy_i) interleaved, same per row
        dv = disp[sl].reshape(1, 2 * NP).view(np.int32)
        aux[:, NP:3 * NP] = dv
        in_maps.append({"fs": t[sl].reshape(TOT), "aux": aux})
    return in_maps, float(mask.sum())


def kernel(**inputs):
    in_maps, cnt = _shard_inputs(inputs)
    nc = _get_program()

    results = run_bass_kernel_spmd(nc, in_maps, core_ids=list(range(NCORES)),
                                   **_RUN_KWARGS)
    globals()["_LAST_RESULTS"] = results

    sums = np.zeros(NF, dtype=np.float32)
    for r in results.results:
        sums += r["out"].reshape(-1).astype(np.float32)

    weights = (np.float32(GAMMA) ** np.arange(NF - 1, -1, -1, dtype=np.float32))
    means = sums / np.float32(cnt)
    loss = np.float32(np.sum(weights * means, dtype=np.float32) * np.float32(LOSS_WEIGHT))
    return np.asarray(loss, dtype=np.float32)


# revision 40
# speedup vs baseline: 1.0174x; 1.0174x over previous
"""KeypointFlowLoss Trainium2 kernel.

The loss only reads each flow at the K keypoint pixels the reference
scatters into the ground-truth image (everywhere else gt == 0, mask == 0),
so instead of streaming 5 x [16,2,512,512] f32 from HBM we gather exactly
the needed pixels with one indirect DMA per core and reduce on-chip.

Sharding: data-parallel over batch — core c owns batches [2c, 2c+2).
Host-side marshalling re-lays the five flows out as one [B,H,W,5,2] tensor
(per-core slice is a contiguous view) so each keypoint's (x,y) pair for
one flow is 8 contiguous bytes, and packs per core one [5,102] i32 aux
block: cols 0-33 the gather's pair-index table (row f = flow f's pair for
each keypoint), cols 34-101 the bitcast f32 keypoint displacements.
Masked-out keypoints get out-of-bounds indices (silently dropped by the
gather, leaving memset zeros) and zero disp, so they contribute exactly 0
to every sum with no mask multiply.

The program is raw bass (no TileContext): hand-placed semaphores, no
entry/exit all-engine drain barriers, which saves ~0.8us of scaffold.
Device critical path per core (everything else overlaps):
  aux DMA -> indirect gather g[5,68] -> d=g-disp, d^2, pair-sum [5,34]
  -> ACT sqrt with free-axis accumulate -> [5,1] partial sums -> out DMA.
The host adds the 8 cores' partials, divides by the host-computed mask
count, and applies the gamma weighting.
"""

import numpy as np

import concourse.bacc as bacc
import concourse.bass as bass
import concourse.mybir as mybir
from concourse.bass import IndirectOffsetOnAxis
from concourse.bass_utils import run_bass_kernel_spmd

B, CH, H, W = 16, 2, 512, 512
K = 17
NF = 5
NCORES = 8
BL = B // NCORES          # batches per core
NP = BL * K               # keypoints per core
NV = NF * CH              # flow values per keypoint
TOT = BL * H * W * NV     # per-core flow elements
GAMMA = 0.8
LOSS_WEIGHT = 1.0

F32 = mybir.dt.float32
I32 = mybir.dt.int32

_PROGRAM = None
_RUN_KWARGS = {}      # test harness can set {"trace": True} to profile
_LAST_RESULTS = None


def _build_program():
    """Raw-bass program (no TileContext): hand-placed semaphores, no entry or
    exit all-engine barriers. Every cross-engine edge is a producer .then_inc
    paired with a consumer wait that also decrements, so all semaphores read
    zero again at program end and the NEFF is safely re-executable."""
    nc = bacc.Bacc(None, target_bir_lowering=False)

    # Strip the constructor's entry all-engine barrier: every cross-engine
    # edge below carries an explicit semaphore, and the const-AP pool the
    # barrier protects is not used (the activation bias is an own memset
    # tile synced through the s_v chain).
    entry = nc.main_func.blocks[0]
    for inst in [i for i in entry.instructions
                 if isinstance(i, (mybir.InstDrain, mybir.InstEventSemaphore))]:
        entry.instructions.remove(inst)

    fs = nc.dram_tensor("fs", [TOT], F32, kind="ExternalInput")
    aux = nc.dram_tensor("aux", [NF, NP], I32, kind="ExternalInput")
    dsp = nc.dram_tensor("dsp", [NF, 2 * NP], F32, kind="ExternalInput")
    out = nc.dram_tensor("out", [NF, 1], F32, kind="ExternalOutput")

    s_rdy = nc.alloc_semaphore("s_rdy")  # aux table (+16) and disp-in-g (+16)
    s_g = nc.alloc_semaphore("s_g")     # gather landed
    s_o = nc.alloc_semaphore("s_o")     # out DMA landed
    s_v = nc.alloc_semaphore("s_v")     # compute chain progress counter
    sems = [s_rdy, s_g, s_o, s_v]

    with (
        nc.sbuf_tensor([NF, NP], I32) as at,
        nc.sbuf_tensor([NF, 2 * NP], F32) as g,
        nc.sbuf_tensor([NF, NP], F32) as s,
        nc.sbuf_tensor([NF, NP], F32) as epe,
        nc.sbuf_tensor([NF, 1], F32) as res,
        nc.sbuf_tensor([NF, 1], F32) as zb,
    ):
        # two parallel input DMAs on separate queues: the gather index table
        # (SP) and the displacements preloaded straight into g (ACT); both
        # bump s_rdy so the gather takes a single wait at 32
        nc.sync.dma_start(out=at[:], in_=aux[:]).then_inc(s_rdy, 16)
        nc.scalar.dma_start(out=g[:], in_=dsp[:]).then_inc(s_rdy, 16)

        nc.vector.memset(zb[:], 0.0)   # activation bias zeros (synced via s_v)

        # gather with compute_op=add: each index fetches the contiguous
        # (x,y) f32 pair of one flow at one keypoint and lands g - disp
        # directly (g was preloaded with -disp; the DGE compute path only
        # supports add). OOB indices (masked keypoints) are dropped, leaving
        # the preloaded zeros (their disp is zeroed host-side), so they
        # contribute 0 with no mask multiply.
        flat = bass.AP(fs, 0, [[2, TOT // 2], [1, 2]])
        nc.gpsimd.indirect_dma_start(
            out=g[:],
            out_offset=None,
            in_=flat,
            in_offset=IndirectOffsetOnAxis(ap=at[:], axis=0),
            bounds_check=TOT // 2 - 1,
            oob_is_err=False,
            compute_op=mybir.AluOpType.add,
        ).wait_op(s_rdy, 32, "sem-ge").then_inc(s_g, 16)

        # col 2i: x-diff of keypoint i; col 2i+1: y-diff
        nc.vector.tensor_tensor(out=g[:], in0=g[:], in1=g[:],
                                op=mybir.AluOpType.mult) \
            .wait_op(s_g, 16, "sem-ge").then_inc(s_v, 1)
        nc.vector.tensor_tensor(out=s[:], in0=g[:, 0:2 * NP:2],
                                in1=g[:, 1:2 * NP:2],
                                op=mybir.AluOpType.add) \
            .wait_op(s_v, 1, "sem-ge").then_inc(s_v, 1)

        # epe = sqrt(s); accum_out gives the per-flow keypoint sum
        nc.scalar.activation(out=epe[:], in_=s[:],
                             func=mybir.ActivationFunctionType.Sqrt,
                             bias=zb[:], accum_out=res[:]) \
            .wait_op(s_v, 2, "sem-ge").then_inc(s_v, 1)

        nc.sync.dma_start(out=out[:], in_=res[:]) \
            .wait_op(s_v, 3, "sem-ge").then_inc(s_o, 16)
        # one barrier so every engine's updates retire, then a single
        # range-clear resets the semaphores for NEFF re-execution
        nc.sync.wait_ge(s_o, 16)
        nc.all_engine_barrier(sem_only=True)
        lo = min(sm.num for sm in sems)
        hi = max(sm.num for sm in sems)
        nc.sync.sem_clear(range(lo, hi + 1))

    nc.finalize()
    return nc


def _get_program():
    global _PROGRAM
    if _PROGRAM is None:
        _PROGRAM = _build_program()
    return _PROGRAM


def _shard_inputs(inputs):
    """Host-side marshalling: returns (in_maps for the 8 cores, mask count)."""
    flows = [np.asarray(inputs[f"flow{i}"], dtype=np.float32) for i in range(NF)]
    kps = np.asarray(inputs["kps"], dtype=np.int64)

    # T[b,y,x,f,c] = flow_f[b,c,y,x]; per-core slice stays a contiguous view.
    t = np.ascontiguousarray(
        np.stack(flows, axis=0).transpose(1, 3, 4, 0, 2)
    ).reshape(B, H * W * NV)

    kps0, kps1 = kps[:, 0], kps[:, 1]        # [B, K, 2] (x, y)
    x0, y0 = kps0[..., 0], kps0[..., 1]
    x1, y1 = kps1[..., 0], kps1[..., 1]
    valid = (
        (kps0 >= 0).all(-1) & (kps1 >= 0).all(-1)
        & (x0 < W) & (y0 < H) & (x1 < W) & (y1 < H)
    )
    disp = (kps1 - kps0).astype(np.float32)  # [B, K, 2]
    mask = valid & (kps1 != kps0).any(-1)    # [B, K]
    disp[~mask] = 0.0

    # pair index of (b, y0, x0)'s first flow pair; OOB when masked out
    idx = np.where(mask, (y0 * W + x0) * NF, TOT).astype(np.int64)   # [B, K]

    in_maps = []
    for c in range(NCORES):
        sl = slice(c * BL, (c + 1) * BL)
        loc = idx[sl] + (np.arange(BL) * (H * W * NF))[:, None]   # [BL, K]
        # pair-index of flow f's (x,y) pair of keypoint i at [f, i]
        f_off = np.arange(NF, dtype=np.int64)[:, None]            # [NF, 1]
        aux = (loc.reshape(1, NP) + f_off).astype(np.int32) \
            + np.zeros((NF, 1), dtype=np.int32)
        # negated disp (dx_i, dy_i) interleaved, replicated across flow rows
        dv = np.broadcast_to(-disp[sl].reshape(1, 2 * NP), (NF, 2 * NP))
        in_maps.append({"fs": t[sl].reshape(TOT), "aux": aux,
                        "dsp": np.ascontiguousarray(dv)})
    return in_maps, float(mask.sum())


def kernel(**inputs):
    in_maps, cnt = _shard_inputs(inputs)
    nc = _get_program()

    results = run_bass_kernel_spmd(nc, in_maps, core_ids=list(range(NCORES)),
                                   **_RUN_KWARGS)
    globals()["_LAST_RESULTS"] = results

    sums = np.zeros(NF, dtype=np.float32)
    for r in results.results:
        sums += r["out"].reshape(-1).astype(np.float32)

    weights = (np.float32(GAMMA) ** np.arange(NF - 1, -1, -1, dtype=np.float32))
    means = sums / np.float32(cnt)
    loss = np.float32(np.sum(weights * means, dtype=np.float32) * np.float32(LOSS_WEIGHT))
    return np.asarray(loss, dtype=np.float32)
